# revision 16
# baseline (speedup 1.0000x reference)
"""Block-circulant SwiGLU feed-forward (CirculantFeedForward) for 8 trn2 cores.

Strategy: token-parallel across the 8 cores (16384 tokens -> 2048/core, no
collectives). Each circulant block is materialized host-side into a dense
matrix, so on-device the whole network is three dense GEMMs + SiLU*mul,
running on the TensorEngine. Host also pre-packs x / weights into the exact
SBUF layouts so every DMA is a contiguous [128, F] transfer.

Per-core geometry (d_model=2048, d_ff=5632, block=512):
  gate/up:  out[m(44x128), tok] += Wgu[k(16x128), m].T @ xT[k, tok]
  h = silu(gate) * up                  (stored bf16/f32 in SBUF)
  down:     out[m2(16x128), tok] += Wd[k2(44x128), m2].T @ h[k2, tok]
Tokens are processed in passes sized to fit SBUF.

PSUM budget (8 banks): two tags "a"/"b" of [128, pass_t] fp32, bufs=2.
gate accumulates in "a", up in "b"; down reuses "a".
"""
import os
from contextlib import ExitStack

import numpy as np
import ml_dtypes

import concourse.bacc as bacc
import concourse.mybir as mybir
import concourse.tile as tile
from concourse.bass_utils import run_bass_kernel_spmd

N_CORES = 8
P = 128
B = 512
D_MODEL = 2048
D_FF = 5632
KT = D_MODEL // P    # 16 k-tiles (gate/up contraction; also down output tiles)
MT = D_FF // P       # 44 m-tiles (gate/up output; down contraction)
TOK_TOTAL = 16384
TOK_CORE = TOK_TOTAL // N_CORES  # 2048

MODE = os.environ.get("BASS_MODE", "bf16")  # bf16 | f32 | f32r

_MODE_CFG = {
    # (mybir dtype, numpy dtype, tokens per pass, matmul N, wgu bufs)
    "bf16": (mybir.dt.bfloat16, ml_dtypes.bfloat16, 1024, 512, 3),
    "f32":  (mybir.dt.float32, np.float32, 512, 512, 2),
    "f32r": (mybir.dt.float32r, np.float32, 512, 512, 2),
}
KCH = 22  # down-weight chunk: MT=44 split into 2 chunks of 22 k-tiles

_built = {}
last_results = None

# ---------------------------------------------------------------------------
# Hartley (DHT) hybrid mode: block-circulant matmul block-diagonalizes in the
# real Hartley basis with (f, B-f) pairs interleaved.  Per 512-block:
#   y_q = iH @ sum_p Mix_qp @ (H @ x_p)
# where Mix_qp is 2x2-block-diagonal => its [128,128] tiles are diagonal.
# PE work per token drops from 2112 to 1184 matmul-tiles.
# ---------------------------------------------------------------------------
HCHUNK = 512   # tokens per chunk in hart mode
N_HCH = TOK_CORE // HCHUNK


def _hartley_mats():
    f = np.arange(B)
    M = np.outer(f, f) * (2 * np.pi / B)
    H = np.cos(M) + np.sin(M)
    order = [0, B // 2] + [v for u in range(1, B // 2) for v in (u, B - u)]
    R = np.array(order)
    T_f = (H[R].T).astype(np.float32)      # [in x, out hx]
    T_i = (H[R] / B).astype(np.float32)    # [in hx, out x]
    return T_f, T_i, R


def _mix_tiles(w, R):
    """w: (q, p, B) -> mix tiles [q, p, 4, 128, 128] ([in-row, out-col])."""
    q, p, _ = w.shape
    W = np.fft.fft(w, axis=-1)
    # DHT of w: W_h[f] = Re(W[f]) - Im(W[f])  (since cas = cos + sin)
    Wh = (W.real - W.imag).astype(np.float64)   # [q, p, B]
    fs = R[0::2]                                 # +side freq of each unit
    gs = R[1::2]
    Wp = (Wh[..., fs] + Wh[..., gs]) / 2         # [q, p, 256]
    Wm = (Wh[..., fs] - Wh[..., gs]) / 2
    # unit 0 is self-paired (f=0, g=B/2): block = diag(Wh[0], Wh[B/2])
    blocks = np.zeros((q, p, B // 2, 2, 2), np.float64)
    blocks[..., 0, 0] = Wp
    blocks[..., 0, 1] = -Wm
    blocks[..., 1, 0] = Wm
    blocks[..., 1, 1] = Wp
    blocks[..., 0, 0, 0] = Wh[..., 0]
    blocks[..., 0, 0, 1] = 0.0
    blocks[..., 0, 1, 0] = 0.0
    blocks[..., 0, 1, 1] = Wh[..., B // 2]
    # assemble [q, p, kt, 128, 128]: 64 units per kt-tile
    tiles = np.zeros((q, p, 4, P, P), np.float32)
    for u_lo in range(4):
        bb = blocks[:, :, u_lo * 64:(u_lo + 1) * 64]      # [q,p,64,2,2]
        t = np.zeros((q, p, 64, 2, 64, 2), np.float64)
        iu = np.arange(64)
        t[:, :, iu, :, iu, :] = bb.transpose(2, 0, 1, 3, 4)
        tiles[:, :, u_lo] = t.reshape(q, p, P, P)
    return tiles


def _build(mode):
    if mode in _built:
        return _built[mode]
    cdt, _, pass_t, mm_n, wgu_bufs = _MODE_CFG[mode]
    n_pass = TOK_CORE // pass_t
    n_nt = pass_t // mm_n  # matmul n-tiles per pass

    nc = bacc.Bacc("TRN2", debug=False, num_devices=N_CORES)
    f32 = mybir.dt.float32

    xT = nc.dram_tensor("xT", [n_pass, P, KT * pass_t], cdt, kind="ExternalInput").ap()
    wgu = nc.dram_tensor("wgu", [MT, P, 2 * KT * P], cdt, kind="ExternalInput").ap()
    wd = nc.dram_tensor("wd", [KT, P, MT * P], cdt, kind="ExternalInput").ap()
    out = nc.dram_tensor("outT", [KT, P, TOK_CORE], f32, kind="ExternalOutput").ap()

    with tile.TileContext(nc) as tc, ExitStack() as ctx:
        xp = ctx.enter_context(tc.tile_pool(name="xp", bufs=1))
        wp = ctx.enter_context(tc.tile_pool(name="wp", bufs=wgu_bufs))
        wdp_ = ctx.enter_context(tc.tile_pool(name="wdp", bufs=2))
        hp = ctx.enter_context(tc.tile_pool(name="hp", bufs=1))
        sp = ctx.enter_context(tc.tile_pool(name="sp", bufs=2))
        op = ctx.enter_context(tc.tile_pool(name="op", bufs=3))
        ps = ctx.enter_context(tc.tile_pool(name="ps", bufs=2, space="PSUM"))

        for ip in range(n_pass):
            x_sb = xp.tile([P, KT, pass_t], cdt, tag="x")
            nc.sync.dma_start(out=x_sb, in_=xT[ip].rearrange("p (kt t) -> p kt t", kt=KT))
            h_sb = hp.tile([P, MT, pass_t], cdt, tag="h")

            # ---- gate/up + SiLU*mul ----
            for m in range(MT):
                w_sb = wp.tile([P, 2, KT, P], cdt, tag="wgu")
                nc.sync.dma_start(
                    out=w_sb, in_=wgu[m].rearrange("p (g kt q) -> p g kt q", g=2, kt=KT)
                )
                pg = ps.tile([P, pass_t], f32, tag="a")
                pu = ps.tile([P, pass_t], f32, tag="b")
                for k in range(KT):
                    for j in range(n_nt):
                        nc.tensor.matmul(
                            pg[:, j * mm_n:(j + 1) * mm_n], w_sb[:, 0, k, :],
                            x_sb[:, k, j * mm_n:(j + 1) * mm_n],
                            start=(k == 0), stop=(k == KT - 1),
                        )
                    for j in range(n_nt):
                        nc.tensor.matmul(
                            pu[:, j * mm_n:(j + 1) * mm_n], w_sb[:, 1, k, :],
                            x_sb[:, k, j * mm_n:(j + 1) * mm_n],
                            start=(k == 0), stop=(k == KT - 1),
                        )
                sg = sp.tile([P, pass_t], f32, tag="sg")
                nc.scalar.activation(sg, pg, mybir.ActivationFunctionType.Silu)
                nc.vector.tensor_mul(h_sb[:, m, :], sg, pu)

            # ---- down ----
            for m2 in range(KT):
                pd = ps.tile([P, pass_t], f32, tag="a")
                for ch in range(MT // KCH):
                    wd_sb = wdp_.tile([P, KCH, P], cdt, tag="wd")
                    nc.sync.dma_start(
                        out=wd_sb,
                        in_=wd[m2][:, ch * KCH * P:(ch + 1) * KCH * P].rearrange(
                            "p (kt q) -> p kt q", kt=KCH
                        ),
                    )
                    for kc in range(KCH):
                        k2 = ch * KCH + kc
                        for j in range(n_nt):
                            nc.tensor.matmul(
                                pd[:, j * mm_n:(j + 1) * mm_n], wd_sb[:, kc, :],
                                h_sb[:, k2, j * mm_n:(j + 1) * mm_n],
                                start=(k2 == 0), stop=(k2 == MT - 1),
                            )
                o_sb = op.tile([P, pass_t], f32, tag="o")
                nc.vector.tensor_copy(o_sb, pd)
                nc.sync.dma_start(
                    out=out[m2][:, ip * pass_t:(ip + 1) * pass_t], in_=o_sb
                )

    nc.finalize()
    _built[mode] = nc
    return nc


def _build_hart():
    if "hart" in _built:
        return _built["hart"]
    cdt = mybir.dt.bfloat16
    f32 = mybir.dt.float32
    nc = bacc.Bacc("TRN2", debug=False, num_devices=N_CORES)

    xT = nc.dram_tensor("xT", [N_HCH, P, KT * HCHUNK], cdt, kind="ExternalInput").ap()
    tf = nc.dram_tensor("tf", [P, 16 * P], cdt, kind="ExternalInput").ap()
    ti = nc.dram_tensor("ti", [P, 16 * P], cdt, kind="ExternalInput").ap()
    mg = nc.dram_tensor("mg", [44, P, 4 * P], cdt, kind="ExternalInput").ap()
    mu = nc.dram_tensor("mu", [44, P, 4 * P], cdt, kind="ExternalInput").ap()
    md = nc.dram_tensor("md", [16, P, 11 * P], cdt, kind="ExternalInput").ap()
    out = nc.dram_tensor("outT", [KT, P, TOK_CORE], f32, kind="ExternalOutput").ap()

    with tile.TileContext(nc) as tc, ExitStack() as ctx:
        xp = ctx.enter_context(tc.tile_pool(name="xp", bufs=2))
        tp = ctx.enter_context(tc.tile_pool(name="tp", bufs=1))
        mwp = ctx.enter_context(tc.tile_pool(name="mwp", bufs=4))
        ap_ = ctx.enter_context(tc.tile_pool(name="ap", bufs=132))
        sp = ctx.enter_context(tc.tile_pool(name="sp", bufs=2))
        op = ctx.enter_context(tc.tile_pool(name="op", bufs=2))
        ps = ctx.enter_context(tc.tile_pool(name="ps", bufs=2, space="PSUM"))
        ps3 = ctx.enter_context(tc.tile_pool(name="ps3", bufs=3, space="PSUM"))
        ps1 = ctx.enter_context(tc.tile_pool(name="ps1", bufs=1, space="PSUM"))

        tf_sb = tp.tile([P, 4, 4, P], cdt, tag="tf")
        ti_sb = tp.tile([P, 4, 4, P], cdt, tag="ti")
        nc.sync.dma_start(out=tf_sb, in_=tf.rearrange("p (a b m) -> p a b m", a=4, b=4))
        nc.sync.dma_start(out=ti_sb, in_=ti.rearrange("p (a b m) -> p a b m", a=4, b=4))

        def load_x(ic):
            t = xp.tile([P, KT, HCHUNK], cdt, tag="x")
            nc.sync.dma_start(out=t, in_=xT[ic].rearrange("p (kt t) -> p kt t", kt=KT))
            return t

        x_next = load_x(0)
        for ic in range(N_HCH):
            x_sb = x_next

            # ---- A: Hx = H @ x per p-block ----
            hx = []
            for p_ in range(4):
                for kt in range(4):
                    pa = ps.tile([P, HCHUNK], f32, tag="ta")
                    for kin in range(4):
                        nc.tensor.matmul(
                            pa, tf_sb[:, kin, kt, :], x_sb[:, p_ * 4 + kin, :],
                            start=(kin == 0), stop=(kin == 3),
                        )
                    t_ = ap_.tile([P, HCHUNK], cdt, tag="act")
                    nc.vector.tensor_copy(t_, pa)
                    hx.append(t_)

            if ic + 1 < N_HCH:
                x_next = load_x(ic + 1)

            # ---- B: mix gate / up ----
            gH, uH = [], []
            for q in range(11):
                for kt in range(4):
                    o = q * 4 + kt
                    wg_sb = mwp.tile([P, 4, P], cdt, tag="mg")
                    nc.sync.dma_start(out=wg_sb, in_=mg[o].rearrange("p (a m) -> p a m", a=4))
                    pb = ps3.tile([P, HCHUNK], f32, tag="tb")
                    for p_ in range(4):
                        nc.tensor.matmul(
                            pb, wg_sb[:, p_, :], hx[p_ * 4 + kt],
                            start=(p_ == 0), stop=(p_ == 3),
                        )
                    t_ = ap_.tile([P, HCHUNK], cdt, tag="act")
                    nc.vector.tensor_copy(t_, pb)
                    gH.append(t_)

                    wu_sb = mwp.tile([P, 4, P], cdt, tag="mg")
                    nc.sync.dma_start(out=wu_sb, in_=mu[o].rearrange("p (a m) -> p a m", a=4))
                    pb2 = ps3.tile([P, HCHUNK], f32, tag="tb")
                    for p_ in range(4):
                        nc.tensor.matmul(
                            pb2, wu_sb[:, p_, :], hx[p_ * 4 + kt],
                            start=(p_ == 0), stop=(p_ == 3),
                        )
                    t2 = ap_.tile([P, HCHUNK], cdt, tag="act")
                    nc.vector.tensor_copy(t2, pb2)
                    uH.append(t2)

            # ---- C: back to real domain, SiLU * up ----
            h = []
            for q in range(11):
                for mt in range(4):
                    pg = ps.tile([P, HCHUNK], f32, tag="tc")
                    for kt in range(4):
                        nc.tensor.matmul(
                            pg, ti_sb[:, kt, mt, :], gH[q * 4 + kt],
                            start=(kt == 0), stop=(kt == 3),
                        )
                    sg = sp.tile([P, HCHUNK], f32, tag="sg")
                    nc.scalar.activation(sg, pg, mybir.ActivationFunctionType.Silu)
                    pu = ps.tile([P, HCHUNK], f32, tag="tc")
                    for kt in range(4):
                        nc.tensor.matmul(
                            pu, ti_sb[:, kt, mt, :], uH[q * 4 + kt],
                            start=(kt == 0), stop=(kt == 3),
                        )
                    t_ = ap_.tile([P, HCHUNK], cdt, tag="act")
                    nc.vector.tensor_mul(t_, sg, pu)
                    h.append(t_)

            # ---- D: Hh = H @ h per q-block ----
            hH = []
            for q in range(11):
                for kt in range(4):
                    pa = ps.tile([P, HCHUNK], f32, tag="ta")
                    for kin in range(4):
                        nc.tensor.matmul(
                            pa, tf_sb[:, kin, kt, :], h[q * 4 + kin],
                            start=(kin == 0), stop=(kin == 3),
                        )
                    t_ = ap_.tile([P, HCHUNK], cdt, tag="act")
                    nc.scalar.copy(t_, pa)
                    hH.append(t_)

            # ---- E: mix down ----
            dH = []
            for p2 in range(4):
                for kt in range(4):
                    o = p2 * 4 + kt
                    wd_sb = mwp.tile([P, 11, P], cdt, tag="md")
                    nc.sync.dma_start(out=wd_sb, in_=md[o].rearrange("p (a m) -> p a m", a=11))
                    pb = ps3.tile([P, HCHUNK], f32, tag="tb")
                    for q in range(11):
                        nc.tensor.matmul(
                            pb, wd_sb[:, q, :], hH[q * 4 + kt],
                            start=(q == 0), stop=(q == 10),
                        )
                    t_ = ap_.tile([P, HCHUNK], cdt, tag="act")
                    nc.vector.tensor_copy(t_, pb)
                    dH.append(t_)

            # ---- F: iH -> real output ----
            for p2 in range(4):
                for mt in range(4):
                    pf = ps1.tile([P, HCHUNK], f32, tag="td")
                    for kt in range(4):
                        nc.tensor.matmul(
                            pf, ti_sb[:, kt, mt, :], dH[p2 * 4 + kt],
                            start=(kt == 0), stop=(kt == 3),
                        )
                    o_sb = op.tile([P, HCHUNK], f32, tag="o")
                    nc.vector.tensor_copy(o_sb, pf)
                    nc.sync.dma_start(
                        out=out[p2 * 4 + mt][:, ic * HCHUNK:(ic + 1) * HCHUNK], in_=o_sb
                    )

    nc.finalize()
    _built["hart"] = nc
    return nc


def _hart_in_maps(x, w_gate, w_up, w_down):
    T_f, T_i, R = _hartley_mats()
    bf = ml_dtypes.bfloat16

    tf_pack = np.ascontiguousarray(
        T_f.reshape(4, P, 4, P).transpose(1, 0, 2, 3)
    ).reshape(P, 16 * P).astype(bf)
    ti_pack = np.ascontiguousarray(
        T_i.reshape(4, P, 4, P).transpose(1, 0, 2, 3)
    ).reshape(P, 16 * P).astype(bf)

    tg = _mix_tiles(np.asarray(w_gate, np.float32), R)   # [11,4,4,128,128]
    tu = _mix_tiles(np.asarray(w_up, np.float32), R)
    td = _mix_tiles(np.asarray(w_down, np.float32), R)   # [4,11,4,128,128]
    mg_pack = np.ascontiguousarray(tg.transpose(0, 2, 3, 1, 4)).reshape(44, P, 4 * P).astype(bf)
    mu_pack = np.ascontiguousarray(tu.transpose(0, 2, 3, 1, 4)).reshape(44, P, 4 * P).astype(bf)
    md_pack = np.ascontiguousarray(td.transpose(0, 2, 3, 1, 4)).reshape(16, P, 11 * P).astype(bf)

    xf = np.asarray(x, np.float32).reshape(TOK_TOTAL, D_MODEL)
    in_maps = []
    for c in range(N_CORES):
        xc = xf[c * TOK_CORE:(c + 1) * TOK_CORE]
        xt = np.ascontiguousarray(
            xc.reshape(N_HCH, HCHUNK, KT, P).transpose(0, 3, 2, 1)
        ).reshape(N_HCH, P, KT * HCHUNK).astype(bf)
        in_maps.append({
            "xT": xt, "tf": tf_pack, "ti": ti_pack,
            "mg": mg_pack, "mu": mu_pack, "md": md_pack,
        })
    return in_maps


def _materialize(w):
    """(q, p, b) circulant generators -> dense [p*b, q*b] (in-dim, out-dim)."""
    q, p, b = w.shape
    i = np.arange(b)
    idx = (i[None, :] - i[:, None]) % b          # [j, i]
    return w[:, :, idx].transpose(1, 2, 0, 3).reshape(p * b, q * b)


def kernel(x, w_gate, w_up, w_down):
    mode = MODE
    if mode == "hart":
        nc = _build_hart()
        in_maps = _hart_in_maps(x, w_gate, w_up, w_down)
        return _run(nc, in_maps)
    cdt, npdt, pass_t, mm_n, _ = _MODE_CFG[mode]
    n_pass = TOK_CORE // pass_t

    nc = _build(mode)

    Wg = _materialize(np.asarray(w_gate, np.float32))   # [2048, 5632]
    Wu = _materialize(np.asarray(w_up, np.float32))     # [2048, 5632]
    Wd = _materialize(np.asarray(w_down, np.float32))   # [5632, 2048]

    # wgu packed: [MT, P, 2, KT, P]; per-partition rows contiguous
    wgu = np.empty((MT, P, 2, KT, P), np.float32)
    wg4 = Wg.reshape(KT, P, MT, P)   # [k, kp, m, mp]
    wu4 = Wu.reshape(KT, P, MT, P)
    wgu[:, :, 0] = wg4.transpose(2, 1, 0, 3)  # [m, kp, k, mp]
    wgu[:, :, 1] = wu4.transpose(2, 1, 0, 3)
    wgu = wgu.reshape(MT, P, 2 * KT * P).astype(npdt)

    wd4 = Wd.reshape(MT, P, KT, P)   # [k2, kp, m2, mp]
    wdp = np.ascontiguousarray(wd4.transpose(2, 1, 0, 3)).reshape(KT, P, MT * P).astype(npdt)

    xf = np.asarray(x, np.float32).reshape(TOK_TOTAL, D_MODEL)
    in_maps = []
    for c in range(N_CORES):
        xc = xf[c * TOK_CORE:(c + 1) * TOK_CORE]          # [2048 tok, 2048 d]
        # -> [n_pass, P, KT, pass_t]: xT[pass, kp, k, t] = xc[pass*pt+t, k*P+kp]
        xt = np.ascontiguousarray(
            xc.reshape(n_pass, pass_t, KT, P).transpose(0, 3, 2, 1)
        ).reshape(n_pass, P, KT * pass_t).astype(npdt)
        in_maps.append({"xT": xt, "wgu": wgu, "wd": wdp})

    return _run(nc, in_maps)


def _run(nc, in_maps):
    trace = bool(os.environ.get("BASS_PROFILE"))
    res = run_bass_kernel_spmd(nc, in_maps, core_ids=list(range(N_CORES)), trace=trace)
    global last_results
    last_results = res

    out = np.empty((TOK_TOTAL, D_MODEL), np.float32)
    for c in range(N_CORES):
        o = res.results[c]["outT"]                         # [KT, P, TOK_CORE]
        out[c * TOK_CORE:(c + 1) * TOK_CORE] = o.reshape(D_MODEL, TOK_CORE).T
    return out.reshape(4, 4096, D_MODEL)


# revision 17
# speedup vs baseline: 1.0625x; 1.0625x over previous
"""Block-circulant SwiGLU feed-forward (CirculantFeedForward) for 8 trn2 cores.

Strategy: token-parallel across the 8 cores (16384 tokens -> 2048/core, no
collectives). Each circulant block is materialized host-side into a dense
matrix, so on-device the whole network is three dense GEMMs + SiLU*mul,
running on the TensorEngine. Host also pre-packs x / weights into the exact
SBUF layouts so every DMA is a contiguous [128, F] transfer.

Per-core geometry (d_model=2048, d_ff=5632, block=512):
  gate/up:  out[m(44x128), tok] += Wgu[k(16x128), m].T @ xT[k, tok]
  h = silu(gate) * up                  (stored bf16/f32 in SBUF)
  down:     out[m2(16x128), tok] += Wd[k2(44x128), m2].T @ h[k2, tok]
Tokens are processed in passes sized to fit SBUF.

PSUM budget (8 banks): two tags "a"/"b" of [128, pass_t] fp32, bufs=2.
gate accumulates in "a", up in "b"; down reuses "a".
"""
import os
from contextlib import ExitStack

import numpy as np
import ml_dtypes

import concourse.bacc as bacc
import concourse.mybir as mybir
import concourse.tile as tile
from concourse.bass_utils import run_bass_kernel_spmd

N_CORES = 8
P = 128
B = 512
D_MODEL = 2048
D_FF = 5632
KT = D_MODEL // P    # 16 k-tiles (gate/up contraction; also down output tiles)
MT = D_FF // P       # 44 m-tiles (gate/up output; down contraction)
TOK_TOTAL = 16384
TOK_CORE = TOK_TOTAL // N_CORES  # 2048

MODE = os.environ.get("BASS_MODE", "bf16")  # bf16 | f32 | f32r

_MODE_CFG = {
    # (mybir dtype, numpy dtype, tokens per pass, matmul N, wgu bufs)
    "bf16": (mybir.dt.bfloat16, ml_dtypes.bfloat16, 1024, 512, 3),
    "f32":  (mybir.dt.float32, np.float32, 512, 512, 2),
    "f32r": (mybir.dt.float32r, np.float32, 512, 512, 2),
}
KCH = 22  # down-weight chunk: MT=44 split into 2 chunks of 22 k-tiles

_built = {}
last_results = None

# ---------------------------------------------------------------------------
# Hartley (DHT) hybrid mode: block-circulant matmul block-diagonalizes in the
# real Hartley basis with (f, B-f) pairs interleaved.  Per 512-block:
#   y_q = iH @ sum_p Mix_qp @ (H @ x_p)
# where Mix_qp is 2x2-block-diagonal => its [128,128] tiles are diagonal.
# PE work per token drops from 2112 to 1184 matmul-tiles.
# ---------------------------------------------------------------------------
HCHUNK = 512   # tokens per chunk in hart mode
N_HCH = TOK_CORE // HCHUNK


def _hartley_mats():
    f = np.arange(B)
    M = np.outer(f, f) * (2 * np.pi / B)
    H = np.cos(M) + np.sin(M)
    order = [0, B // 2] + [v for u in range(1, B // 2) for v in (u, B - u)]
    R = np.array(order)
    T_f = (H[R].T).astype(np.float32)      # [in x, out hx]
    T_i = (H[R] / B).astype(np.float32)    # [in hx, out x]
    return T_f, T_i, R


def _mix_tiles(w, R):
    """w: (q, p, B) -> mix tiles [q, p, 4, 128, 128] ([in-row, out-col])."""
    q, p, _ = w.shape
    W = np.fft.fft(w, axis=-1)
    # DHT of w: W_h[f] = Re(W[f]) - Im(W[f])  (since cas = cos + sin)
    Wh = (W.real - W.imag).astype(np.float64)   # [q, p, B]
    fs = R[0::2]                                 # +side freq of each unit
    gs = R[1::2]
    Wp = (Wh[..., fs] + Wh[..., gs]) / 2         # [q, p, 256]
    Wm = (Wh[..., fs] - Wh[..., gs]) / 2
    # unit 0 is self-paired (f=0, g=B/2): block = diag(Wh[0], Wh[B/2])
    blocks = np.zeros((q, p, B // 2, 2, 2), np.float64)
    blocks[..., 0, 0] = Wp
    blocks[..., 0, 1] = -Wm
    blocks[..., 1, 0] = Wm
    blocks[..., 1, 1] = Wp
    blocks[..., 0, 0, 0] = Wh[..., 0]
    blocks[..., 0, 0, 1] = 0.0
    blocks[..., 0, 1, 0] = 0.0
    blocks[..., 0, 1, 1] = Wh[..., B // 2]
    # assemble [q, p, kt, 128, 128]: 64 units per kt-tile
    tiles = np.zeros((q, p, 4, P, P), np.float32)
    for u_lo in range(4):
        bb = blocks[:, :, u_lo * 64:(u_lo + 1) * 64]      # [q,p,64,2,2]
        t = np.zeros((q, p, 64, 2, 64, 2), np.float64)
        iu = np.arange(64)
        t[:, :, iu, :, iu, :] = bb.transpose(2, 0, 1, 3, 4)
        tiles[:, :, u_lo] = t.reshape(q, p, P, P)
    return tiles


def _build(mode):
    if mode in _built:
        return _built[mode]
    cdt, _, pass_t, mm_n, wgu_bufs = _MODE_CFG[mode]
    n_pass = TOK_CORE // pass_t
    n_nt = pass_t // mm_n  # matmul n-tiles per pass

    nc = bacc.Bacc("TRN2", debug=False, num_devices=N_CORES)
    f32 = mybir.dt.float32

    xT = nc.dram_tensor("xT", [n_pass, P, KT * pass_t], cdt, kind="ExternalInput").ap()
    wgu = nc.dram_tensor("wgu", [MT, P, 2 * KT * P], cdt, kind="ExternalInput").ap()
    wd = nc.dram_tensor("wd", [KT, P, MT * P], cdt, kind="ExternalInput").ap()
    out = nc.dram_tensor("outT", [KT, P, TOK_CORE], f32, kind="ExternalOutput").ap()

    with tile.TileContext(nc) as tc, ExitStack() as ctx:
        xp = ctx.enter_context(tc.tile_pool(name="xp", bufs=1))
        wp = ctx.enter_context(tc.tile_pool(name="wp", bufs=wgu_bufs))
        wdp_ = ctx.enter_context(tc.tile_pool(name="wdp", bufs=2))
        hp = ctx.enter_context(tc.tile_pool(name="hp", bufs=1))
        sp = ctx.enter_context(tc.tile_pool(name="sp", bufs=2))
        op = ctx.enter_context(tc.tile_pool(name="op", bufs=3))
        ps = ctx.enter_context(tc.tile_pool(name="ps", bufs=2, space="PSUM"))

        for ip in range(n_pass):
            x_sb = xp.tile([P, KT, pass_t], cdt, tag="x")
            nc.sync.dma_start(out=x_sb, in_=xT[ip].rearrange("p (kt t) -> p kt t", kt=KT))
            h_sb = hp.tile([P, MT, pass_t], cdt, tag="h")

            # ---- gate/up + SiLU*mul ----
            for m in range(MT):
                w_sb = wp.tile([P, 2, KT, P], cdt, tag="wgu")
                nc.sync.dma_start(
                    out=w_sb, in_=wgu[m].rearrange("p (g kt q) -> p g kt q", g=2, kt=KT)
                )
                pg = ps.tile([P, pass_t], f32, tag="a")
                pu = ps.tile([P, pass_t], f32, tag="b")
                for k in range(KT):
                    for j in range(n_nt):
                        nc.tensor.matmul(
                            pg[:, j * mm_n:(j + 1) * mm_n], w_sb[:, 0, k, :],
                            x_sb[:, k, j * mm_n:(j + 1) * mm_n],
                            start=(k == 0), stop=(k == KT - 1),
                        )
                    for j in range(n_nt):
                        nc.tensor.matmul(
                            pu[:, j * mm_n:(j + 1) * mm_n], w_sb[:, 1, k, :],
                            x_sb[:, k, j * mm_n:(j + 1) * mm_n],
                            start=(k == 0), stop=(k == KT - 1),
                        )
                sg = sp.tile([P, pass_t], f32, tag="sg")
                nc.scalar.activation(sg, pg, mybir.ActivationFunctionType.Silu)
                nc.vector.tensor_mul(h_sb[:, m, :], sg, pu)

            # ---- down ----
            for m2 in range(KT):
                pd = ps.tile([P, pass_t], f32, tag="a")
                for ch in range(MT // KCH):
                    wd_sb = wdp_.tile([P, KCH, P], cdt, tag="wd")
                    nc.sync.dma_start(
                        out=wd_sb,
                        in_=wd[m2][:, ch * KCH * P:(ch + 1) * KCH * P].rearrange(
                            "p (kt q) -> p kt q", kt=KCH
                        ),
                    )
                    for kc in range(KCH):
                        k2 = ch * KCH + kc
                        for j in range(n_nt):
                            nc.tensor.matmul(
                                pd[:, j * mm_n:(j + 1) * mm_n], wd_sb[:, kc, :],
                                h_sb[:, k2, j * mm_n:(j + 1) * mm_n],
                                start=(k2 == 0), stop=(k2 == MT - 1),
                            )
                o_sb = op.tile([P, pass_t], f32, tag="o")
                nc.vector.tensor_copy(o_sb, pd)
                nc.sync.dma_start(
                    out=out[m2][:, ip * pass_t:(ip + 1) * pass_t], in_=o_sb
                )

    nc.finalize()
    _built[mode] = nc
    return nc


def _build_hart():
    if "hart" in _built:
        return _built["hart"]
    cdt = mybir.dt.bfloat16
    f32 = mybir.dt.float32
    nc = bacc.Bacc("TRN2", debug=False, num_devices=N_CORES)

    xT = nc.dram_tensor("xT", [N_HCH, P, KT * HCHUNK], cdt, kind="ExternalInput").ap()
    tf = nc.dram_tensor("tf", [P, 16 * P], cdt, kind="ExternalInput").ap()
    ti = nc.dram_tensor("ti", [P, 16 * P], cdt, kind="ExternalInput").ap()
    mg = nc.dram_tensor("mg", [44, P, 4 * P], cdt, kind="ExternalInput").ap()
    mu = nc.dram_tensor("mu", [44, P, 4 * P], cdt, kind="ExternalInput").ap()
    md = nc.dram_tensor("md", [16, P, 11 * P], cdt, kind="ExternalInput").ap()
    out = nc.dram_tensor("outT", [KT, P, TOK_CORE], f32, kind="ExternalOutput").ap()

    with tile.TileContext(nc) as tc, ExitStack() as ctx:
        xp = ctx.enter_context(tc.tile_pool(name="xp", bufs=2))
        tp = ctx.enter_context(tc.tile_pool(name="tp", bufs=1))
        mwp = ctx.enter_context(tc.tile_pool(name="mwp", bufs=4))
        ap_ = ctx.enter_context(tc.tile_pool(name="ap", bufs=132))
        sp = ctx.enter_context(tc.tile_pool(name="sp", bufs=2))
        op = ctx.enter_context(tc.tile_pool(name="op", bufs=2))
        ps = ctx.enter_context(tc.tile_pool(name="ps", bufs=2, space="PSUM"))
        ps3 = ctx.enter_context(tc.tile_pool(name="ps3", bufs=3, space="PSUM"))
        ps1 = ctx.enter_context(tc.tile_pool(name="ps1", bufs=1, space="PSUM"))

        tf_sb = tp.tile([P, 4, 4, P], cdt, tag="tf")
        ti_sb = tp.tile([P, 4, 4, P], cdt, tag="ti")
        nc.sync.dma_start(out=tf_sb, in_=tf.rearrange("p (a b m) -> p a b m", a=4, b=4))
        nc.sync.dma_start(out=ti_sb, in_=ti.rearrange("p (a b m) -> p a b m", a=4, b=4))

        def load_x(ic):
            t = xp.tile([P, KT, HCHUNK], cdt, tag="x")
            nc.sync.dma_start(out=t, in_=xT[ic].rearrange("p (kt t) -> p kt t", kt=KT))
            return t

        x_next = load_x(0)
        for ic in range(N_HCH):
            x_sb = x_next

            # ---- A: Hx = H @ x per p-block ----
            hx = []
            for p_ in range(4):
                for kt in range(4):
                    pa = ps.tile([P, HCHUNK], f32, tag="ta")
                    for kin in range(4):
                        nc.tensor.matmul(
                            pa, tf_sb[:, kin, kt, :], x_sb[:, p_ * 4 + kin, :],
                            start=(kin == 0), stop=(kin == 3),
                        )
                    t_ = ap_.tile([P, HCHUNK], cdt, tag="act")
                    nc.vector.tensor_copy(t_, pa)
                    hx.append(t_)

            if ic + 1 < N_HCH:
                x_next = load_x(ic + 1)

            # ---- B: mix gate / up ----
            gH, uH = [], []
            for q in range(11):
                for kt in range(4):
                    o = q * 4 + kt
                    wg_sb = mwp.tile([P, 4, P], cdt, tag="mg")
                    nc.sync.dma_start(out=wg_sb, in_=mg[o].rearrange("p (a m) -> p a m", a=4))
                    pb = ps.tile([P, HCHUNK], f32, tag="tb")
                    for p_ in range(4):
                        nc.tensor.matmul(
                            pb, wg_sb[:, p_, :], hx[p_ * 4 + kt],
                            start=(p_ == 0), stop=(p_ == 3),
                        )
                    t_ = ap_.tile([P, HCHUNK], cdt, tag="act")
                    nc.vector.tensor_copy(t_, pb)
                    gH.append(t_)

                    wu_sb = mwp.tile([P, 4, P], cdt, tag="mg")
                    nc.sync.dma_start(out=wu_sb, in_=mu[o].rearrange("p (a m) -> p a m", a=4))
                    pb2 = ps.tile([P, HCHUNK], f32, tag="tb")
                    for p_ in range(4):
                        nc.tensor.matmul(
                            pb2, wu_sb[:, p_, :], hx[p_ * 4 + kt],
                            start=(p_ == 0), stop=(p_ == 3),
                        )
                    t2 = ap_.tile([P, HCHUNK], cdt, tag="act")
                    nc.vector.tensor_copy(t2, pb2)
                    uH.append(t2)

            # ---- C: back to real domain, SiLU * up ----
            h = []
            for q in range(11):
                for mt in range(4):
                    pg = ps.tile([P, HCHUNK], f32, tag="tc")
                    for kt in range(4):
                        nc.tensor.matmul(
                            pg, ti_sb[:, kt, mt, :], gH[q * 4 + kt],
                            start=(kt == 0), stop=(kt == 3),
                        )
                    sg = sp.tile([P, HCHUNK], f32, tag="sg")
                    nc.scalar.activation(sg, pg, mybir.ActivationFunctionType.Silu)
                    pu = ps.tile([P, HCHUNK], f32, tag="tc")
                    for kt in range(4):
                        nc.tensor.matmul(
                            pu, ti_sb[:, kt, mt, :], uH[q * 4 + kt],
                            start=(kt == 0), stop=(kt == 3),
                        )
                    t_ = ap_.tile([P, HCHUNK], cdt, tag="act")
                    nc.vector.tensor_mul(t_, sg, pu)
                    h.append(t_)

            # ---- D: Hh = H @ h per q-block ----
            hH = []
            for q in range(11):
                for kt in range(4):
                    pa = ps.tile([P, HCHUNK], f32, tag="ta")
                    for kin in range(4):
                        nc.tensor.matmul(
                            pa, tf_sb[:, kin, kt, :], h[q * 4 + kin],
                            start=(kin == 0), stop=(kin == 3),
                        )
                    t_ = ap_.tile([P, HCHUNK], cdt, tag="act")
                    if q % 2 == 0:
                        nc.scalar.copy(t_, pa)
                    else:
                        nc.vector.tensor_copy(t_, pa)
                    hH.append(t_)

            # ---- E: mix down ----
            dH = []
            for p2 in range(4):
                for kt in range(4):
                    o = p2 * 4 + kt
                    wd_sb = mwp.tile([P, 11, P], cdt, tag="md")
                    nc.sync.dma_start(out=wd_sb, in_=md[o].rearrange("p (a m) -> p a m", a=11))
                    pb = ps.tile([P, HCHUNK], f32, tag="tb")
                    for q in range(11):
                        nc.tensor.matmul(
                            pb, wd_sb[:, q, :], hH[q * 4 + kt],
                            start=(q == 0), stop=(q == 10),
                        )
                    t_ = ap_.tile([P, HCHUNK], cdt, tag="act")
                    nc.vector.tensor_copy(t_, pb)
                    dH.append(t_)

            # ---- F: iH -> real output ----
            for p2 in range(4):
                for mt in range(4):
                    pf = ps.tile([P, HCHUNK], f32, tag="td")
                    for kt in range(4):
                        nc.tensor.matmul(
                            pf, ti_sb[:, kt, mt, :], dH[p2 * 4 + kt],
                            start=(kt == 0), stop=(kt == 3),
                        )
                    o_sb = op.tile([P, HCHUNK], f32, tag="o")
                    nc.vector.tensor_copy(o_sb, pf)
                    nc.sync.dma_start(
                        out=out[p2 * 4 + mt][:, ic * HCHUNK:(ic + 1) * HCHUNK], in_=o_sb
                    )

    nc.finalize()
    _built["hart"] = nc
    return nc


def _hart_in_maps(x, w_gate, w_up, w_down):
    T_f, T_i, R = _hartley_mats()
    bf = ml_dtypes.bfloat16

    tf_pack = np.ascontiguousarray(
        T_f.reshape(4, P, 4, P).transpose(1, 0, 2, 3)
    ).reshape(P, 16 * P).astype(bf)
    ti_pack = np.ascontiguousarray(
        T_i.reshape(4, P, 4, P).transpose(1, 0, 2, 3)
    ).reshape(P, 16 * P).astype(bf)

    tg = _mix_tiles(np.asarray(w_gate, np.float32), R)   # [11,4,4,128,128]
    tu = _mix_tiles(np.asarray(w_up, np.float32), R)
    td = _mix_tiles(np.asarray(w_down, np.float32), R)   # [4,11,4,128,128]
    mg_pack = np.ascontiguousarray(tg.transpose(0, 2, 3, 1, 4)).reshape(44, P, 4 * P).astype(bf)
    mu_pack = np.ascontiguousarray(tu.transpose(0, 2, 3, 1, 4)).reshape(44, P, 4 * P).astype(bf)
    md_pack = np.ascontiguousarray(td.transpose(0, 2, 3, 1, 4)).reshape(16, P, 11 * P).astype(bf)

    xf = np.asarray(x, np.float32).reshape(TOK_TOTAL, D_MODEL)
    in_maps = []
    for c in range(N_CORES):
        xc = xf[c * TOK_CORE:(c + 1) * TOK_CORE]
        xt = np.ascontiguousarray(
            xc.reshape(N_HCH, HCHUNK, KT, P).transpose(0, 3, 2, 1)
        ).reshape(N_HCH, P, KT * HCHUNK).astype(bf)
        in_maps.append({
            "xT": xt, "tf": tf_pack, "ti": ti_pack,
            "mg": mg_pack, "mu": mu_pack, "md": md_pack,
        })
    return in_maps


def _materialize(w):
    """(q, p, b) circulant generators -> dense [p*b, q*b] (in-dim, out-dim)."""
    q, p, b = w.shape
    i = np.arange(b)
    idx = (i[None, :] - i[:, None]) % b          # [j, i]
    return w[:, :, idx].transpose(1, 2, 0, 3).reshape(p * b, q * b)


def kernel(x, w_gate, w_up, w_down):
    mode = MODE
    if mode == "hart":
        nc = _build_hart()
        in_maps = _hart_in_maps(x, w_gate, w_up, w_down)
        return _run(nc, in_maps)
    cdt, npdt, pass_t, mm_n, _ = _MODE_CFG[mode]
    n_pass = TOK_CORE // pass_t

    nc = _build(mode)

    Wg = _materialize(np.asarray(w_gate, np.float32))   # [2048, 5632]
    Wu = _materialize(np.asarray(w_up, np.float32))     # [2048, 5632]
    Wd = _materialize(np.asarray(w_down, np.float32))   # [5632, 2048]

    # wgu packed: [MT, P, 2, KT, P]; per-partition rows contiguous
    wgu = np.empty((MT, P, 2, KT, P), np.float32)
    wg4 = Wg.reshape(KT, P, MT, P)   # [k, kp, m, mp]
    wu4 = Wu.reshape(KT, P, MT, P)
    wgu[:, :, 0] = wg4.transpose(2, 1, 0, 3)  # [m, kp, k, mp]
    wgu[:, :, 1] = wu4.transpose(2, 1, 0, 3)
    wgu = wgu.reshape(MT, P, 2 * KT * P).astype(npdt)

    wd4 = Wd.reshape(MT, P, KT, P)   # [k2, kp, m2, mp]
    wdp = np.ascontiguousarray(wd4.transpose(2, 1, 0, 3)).reshape(KT, P, MT * P).astype(npdt)

    xf = np.asarray(x, np.float32).reshape(TOK_TOTAL, D_MODEL)
    in_maps = []
    for c in range(N_CORES):
        xc = xf[c * TOK_CORE:(c + 1) * TOK_CORE]          # [2048 tok, 2048 d]
        # -> [n_pass, P, KT, pass_t]: xT[pass, kp, k, t] = xc[pass*pt+t, k*P+kp]
        xt = np.ascontiguousarray(
            xc.reshape(n_pass, pass_t, KT, P).transpose(0, 3, 2, 1)
        ).reshape(n_pass, P, KT * pass_t).astype(npdt)
        in_maps.append({"xT": xt, "wgu": wgu, "wd": wdp})

    return _run(nc, in_maps)


def _run(nc, in_maps):
    trace = bool(os.environ.get("BASS_PROFILE"))
    res = run_bass_kernel_spmd(nc, in_maps, core_ids=list(range(N_CORES)), trace=trace)
    global last_results
    last_results = res

    out = np.empty((TOK_TOTAL, D_MODEL), np.float32)
    for c in range(N_CORES):
        o = res.results[c]["outT"]                         # [KT, P, TOK_CORE]
        out[c * TOK_CORE:(c + 1) * TOK_CORE] = o.reshape(D_MODEL, TOK_CORE).T
    return out.reshape(4, 4096, D_MODEL)


# revision 19
# speedup vs baseline: 1.0663x; 1.0036x over previous
"""Block-circulant SwiGLU feed-forward (CirculantFeedForward) for 8 trn2 cores.

Sharding: token-parallel across the 8 cores (16384 tokens -> 2048/core, no
collectives).  All weights are tiny circulant generators; host-side prep
turns them into dense matmul tiles and packs every operand into the exact
SBUF layout so each DMA is a contiguous [128, F] transfer.

Default mode "hart" exploits the circulant structure: a block-circulant
matmul block-diagonalizes in the real Hartley (DHT) basis with (f, B-f)
pairs interleaved, so each layer becomes
  y_q = iH @ sum_p Mix_qp @ (H @ x_p)
where the Mix_qp [128,128] tiles are diagonal-only.  PE work per token
drops from 2112 to 1184 [128x128xN] matmul tiles (measured ~1.16 ms vs
~1.86 ms for the dense-materialized variant, rel err 8.1e-3 vs 4.3e-3 in
bf16).  Mode "bf16" is the dense-materialized fallback; "f32r"/"f32" are
higher-precision fallbacks (f32r: rel err 2.6e-4, ~2.6 ms).
"""
import os
from contextlib import ExitStack

import numpy as np
import ml_dtypes

import concourse.bacc as bacc
import concourse.mybir as mybir
import concourse.tile as tile
from concourse.bass_utils import run_bass_kernel_spmd

N_CORES = 8
P = 128
B = 512
D_MODEL = 2048
D_FF = 5632
KT = D_MODEL // P    # 16 k-tiles (gate/up contraction; also down output tiles)
MT = D_FF // P       # 44 m-tiles (gate/up output; down contraction)
TOK_TOTAL = 16384
TOK_CORE = TOK_TOTAL // N_CORES  # 2048

MODE = os.environ.get("BASS_MODE", "hart")  # hart | bf16 | f32 | f32r

_MODE_CFG = {
    # (mybir dtype, numpy dtype, tokens per pass, matmul N, wgu bufs)
    "bf16": (mybir.dt.bfloat16, ml_dtypes.bfloat16, 1024, 512, 3),
    "f32":  (mybir.dt.float32, np.float32, 512, 512, 2),
    "f32r": (mybir.dt.float32r, np.float32, 512, 512, 2),
}
KCH = 22  # down-weight chunk: MT=44 split into 2 chunks of 22 k-tiles

_built = {}
last_results = None

# ---------------------------------------------------------------------------
# Hartley (DHT) hybrid mode: block-circulant matmul block-diagonalizes in the
# real Hartley basis with (f, B-f) pairs interleaved.  Per 512-block:
#   y_q = iH @ sum_p Mix_qp @ (H @ x_p)
# where Mix_qp is 2x2-block-diagonal => its [128,128] tiles are diagonal.
# PE work per token drops from 2112 to 1184 matmul-tiles.
# ---------------------------------------------------------------------------
HCHUNK = 512   # tokens per chunk in hart mode
N_HCH = TOK_CORE // HCHUNK


def _hartley_mats():
    f = np.arange(B)
    M = np.outer(f, f) * (2 * np.pi / B)
    H = np.cos(M) + np.sin(M)
    order = [0, B // 2] + [v for u in range(1, B // 2) for v in (u, B - u)]
    R = np.array(order)
    T_f = (H[R].T).astype(np.float32)      # [in x, out hx]
    T_i = (H[R] / B).astype(np.float32)    # [in hx, out x]
    return T_f, T_i, R


def _mix_tiles(w, R):
    """w: (q, p, B) -> mix tiles [q, p, 4, 128, 128] ([in-row, out-col])."""
    q, p, _ = w.shape
    W = np.fft.fft(w, axis=-1)
    # DHT of w: W_h[f] = Re(W[f]) - Im(W[f])  (since cas = cos + sin)
    Wh = (W.real - W.imag).astype(np.float64)   # [q, p, B]
    fs = R[0::2]                                 # +side freq of each unit
    gs = R[1::2]
    Wp = (Wh[..., fs] + Wh[..., gs]) / 2         # [q, p, 256]
    Wm = (Wh[..., fs] - Wh[..., gs]) / 2
    # unit 0 is self-paired (f=0, g=B/2): block = diag(Wh[0], Wh[B/2])
    blocks = np.zeros((q, p, B // 2, 2, 2), np.float64)
    blocks[..., 0, 0] = Wp
    blocks[..., 0, 1] = -Wm
    blocks[..., 1, 0] = Wm
    blocks[..., 1, 1] = Wp
    blocks[..., 0, 0, 0] = Wh[..., 0]
    blocks[..., 0, 0, 1] = 0.0
    blocks[..., 0, 1, 0] = 0.0
    blocks[..., 0, 1, 1] = Wh[..., B // 2]
    # assemble [q, p, kt, 128, 128]: 64 units per kt-tile
    tiles = np.zeros((q, p, 4, P, P), np.float32)
    for u_lo in range(4):
        bb = blocks[:, :, u_lo * 64:(u_lo + 1) * 64]      # [q,p,64,2,2]
        t = np.zeros((q, p, 64, 2, 64, 2), np.float64)
        iu = np.arange(64)
        t[:, :, iu, :, iu, :] = bb.transpose(2, 0, 1, 3, 4)
        tiles[:, :, u_lo] = t.reshape(q, p, P, P)
    return tiles


def _build(mode):
    if mode in _built:
        return _built[mode]
    cdt, _, pass_t, mm_n, wgu_bufs = _MODE_CFG[mode]
    n_pass = TOK_CORE // pass_t
    n_nt = pass_t // mm_n  # matmul n-tiles per pass

    nc = bacc.Bacc("TRN2", debug=False, num_devices=N_CORES)
    f32 = mybir.dt.float32

    xT = nc.dram_tensor("xT", [n_pass, P, KT * pass_t], cdt, kind="ExternalInput").ap()
    wgu = nc.dram_tensor("wgu", [MT, P, 2 * KT * P], cdt, kind="ExternalInput").ap()
    wd = nc.dram_tensor("wd", [KT, P, MT * P], cdt, kind="ExternalInput").ap()
    out = nc.dram_tensor("outT", [KT, P, TOK_CORE], f32, kind="ExternalOutput").ap()

    with tile.TileContext(nc) as tc, ExitStack() as ctx:
        xp = ctx.enter_context(tc.tile_pool(name="xp", bufs=1))
        wp = ctx.enter_context(tc.tile_pool(name="wp", bufs=wgu_bufs))
        wdp_ = ctx.enter_context(tc.tile_pool(name="wdp", bufs=2))
        hp = ctx.enter_context(tc.tile_pool(name="hp", bufs=1))
        sp = ctx.enter_context(tc.tile_pool(name="sp", bufs=2))
        op = ctx.enter_context(tc.tile_pool(name="op", bufs=3))
        ps = ctx.enter_context(tc.tile_pool(name="ps", bufs=2, space="PSUM"))

        for ip in range(n_pass):
            x_sb = xp.tile([P, KT, pass_t], cdt, tag="x")
            nc.sync.dma_start(out=x_sb, in_=xT[ip].rearrange("p (kt t) -> p kt t", kt=KT))
            h_sb = hp.tile([P, MT, pass_t], cdt, tag="h")

            # ---- gate/up + SiLU*mul ----
            for m in range(MT):
                w_sb = wp.tile([P, 2, KT, P], cdt, tag="wgu")
                nc.sync.dma_start(
                    out=w_sb, in_=wgu[m].rearrange("p (g kt q) -> p g kt q", g=2, kt=KT)
                )
                pg = ps.tile([P, pass_t], f32, tag="a")
                pu = ps.tile([P, pass_t], f32, tag="b")
                for k in range(KT):
                    for j in range(n_nt):
                        nc.tensor.matmul(
                            pg[:, j * mm_n:(j + 1) * mm_n], w_sb[:, 0, k, :],
                            x_sb[:, k, j * mm_n:(j + 1) * mm_n],
                            start=(k == 0), stop=(k == KT - 1),
                        )
                    for j in range(n_nt):
                        nc.tensor.matmul(
                            pu[:, j * mm_n:(j + 1) * mm_n], w_sb[:, 1, k, :],
                            x_sb[:, k, j * mm_n:(j + 1) * mm_n],
                            start=(k == 0), stop=(k == KT - 1),
                        )
                sg = sp.tile([P, pass_t], f32, tag="sg")
                nc.scalar.activation(sg, pg, mybir.ActivationFunctionType.Silu)
                nc.vector.tensor_mul(h_sb[:, m, :], sg, pu)

            # ---- down ----
            for m2 in range(KT):
                pd = ps.tile([P, pass_t], f32, tag="a")
                for ch in range(MT // KCH):
                    wd_sb = wdp_.tile([P, KCH, P], cdt, tag="wd")
                    nc.sync.dma_start(
                        out=wd_sb,
                        in_=wd[m2][:, ch * KCH * P:(ch + 1) * KCH * P].rearrange(
                            "p (kt q) -> p kt q", kt=KCH
                        ),
                    )
                    for kc in range(KCH):
                        k2 = ch * KCH + kc
                        for j in range(n_nt):
                            nc.tensor.matmul(
                                pd[:, j * mm_n:(j + 1) * mm_n], wd_sb[:, kc, :],
                                h_sb[:, k2, j * mm_n:(j + 1) * mm_n],
                                start=(k2 == 0), stop=(k2 == MT - 1),
                            )
                o_sb = op.tile([P, pass_t], f32, tag="o")
                nc.vector.tensor_copy(o_sb, pd)
                nc.sync.dma_start(
                    out=out[m2][:, ip * pass_t:(ip + 1) * pass_t], in_=o_sb
                )

    nc.finalize()
    _built[mode] = nc
    return nc


def _build_hart():
    if "hart" in _built:
        return _built["hart"]
    cdt = mybir.dt.bfloat16
    f32 = mybir.dt.float32
    nc = bacc.Bacc("TRN2", debug=False, num_devices=N_CORES)

    xT = nc.dram_tensor("xT", [N_HCH, P, KT * HCHUNK], cdt, kind="ExternalInput").ap()
    tf = nc.dram_tensor("tf", [P, 16 * P], cdt, kind="ExternalInput").ap()
    ti = nc.dram_tensor("ti", [P, 16 * P], cdt, kind="ExternalInput").ap()
    mg = nc.dram_tensor("mg", [44, P, 4 * P], cdt, kind="ExternalInput").ap()
    mu = nc.dram_tensor("mu", [44, P, 4 * P], cdt, kind="ExternalInput").ap()
    md = nc.dram_tensor("md", [16, P, 11 * P], cdt, kind="ExternalInput").ap()
    out = nc.dram_tensor("outT", [KT, P, TOK_CORE], f32, kind="ExternalOutput").ap()

    with tile.TileContext(nc) as tc, ExitStack() as ctx:
        xp = ctx.enter_context(tc.tile_pool(name="xp", bufs=2))
        tp = ctx.enter_context(tc.tile_pool(name="tp", bufs=1))
        mwp = ctx.enter_context(tc.tile_pool(name="mwp", bufs=4))
        ap_ = ctx.enter_context(tc.tile_pool(name="ap", bufs=132))
        sp = ctx.enter_context(tc.tile_pool(name="sp", bufs=2))
        op = ctx.enter_context(tc.tile_pool(name="op", bufs=2))
        ps = ctx.enter_context(tc.tile_pool(name="ps", bufs=2, space="PSUM"))
        ps3 = ctx.enter_context(tc.tile_pool(name="ps3", bufs=3, space="PSUM"))
        ps1 = ctx.enter_context(tc.tile_pool(name="ps1", bufs=1, space="PSUM"))

        tf_sb = tp.tile([P, 4, 4, P], cdt, tag="tf")
        ti_sb = tp.tile([P, 4, 4, P], cdt, tag="ti")
        nc.sync.dma_start(out=tf_sb, in_=tf.rearrange("p (a b m) -> p a b m", a=4, b=4))
        nc.sync.dma_start(out=ti_sb, in_=ti.rearrange("p (a b m) -> p a b m", a=4, b=4))

        def load_x(ic):
            t = xp.tile([P, KT, HCHUNK], cdt, tag="x")
            nc.sync.dma_start(out=t, in_=xT[ic].rearrange("p (kt t) -> p kt t", kt=KT))
            return t

        x_next = load_x(0)
        for ic in range(N_HCH):
            x_sb = x_next

            # ---- A: Hx = H @ x per p-block ----
            hx = []
            for p_ in range(4):
                for kt in range(4):
                    pa = ps.tile([P, HCHUNK], f32, tag="ta")
                    for kin in range(4):
                        nc.tensor.matmul(
                            pa, tf_sb[:, kin, kt, :], x_sb[:, p_ * 4 + kin, :],
                            start=(kin == 0), stop=(kin == 3),
                        )
                    t_ = ap_.tile([P, HCHUNK], cdt, tag="act")
                    nc.vector.tensor_copy(t_, pa)
                    hx.append(t_)

            if ic + 1 < N_HCH:
                x_next = load_x(ic + 1)

            # ---- B: mix gate / up ----
            gH, uH = [], []
            for q in range(11):
                for kt in range(4):
                    o = q * 4 + kt
                    wg_sb = mwp.tile([P, 4, P], cdt, tag="mg")
                    nc.sync.dma_start(out=wg_sb, in_=mg[o].rearrange("p (a m) -> p a m", a=4))
                    pb = ps.tile([P, HCHUNK], f32, tag="tb")
                    for p_ in range(4):
                        nc.tensor.matmul(
                            pb, wg_sb[:, p_, :], hx[p_ * 4 + kt],
                            start=(p_ == 0), stop=(p_ == 3),
                        )
                    t_ = ap_.tile([P, HCHUNK], cdt, tag="act")
                    nc.vector.tensor_copy(t_, pb)
                    gH.append(t_)

                    wu_sb = mwp.tile([P, 4, P], cdt, tag="mg")
                    nc.sync.dma_start(out=wu_sb, in_=mu[o].rearrange("p (a m) -> p a m", a=4))
                    pb2 = ps.tile([P, HCHUNK], f32, tag="tb")
                    for p_ in range(4):
                        nc.tensor.matmul(
                            pb2, wu_sb[:, p_, :], hx[p_ * 4 + kt],
                            start=(p_ == 0), stop=(p_ == 3),
                        )
                    t2 = ap_.tile([P, HCHUNK], cdt, tag="act")
                    nc.vector.tensor_copy(t2, pb2)
                    uH.append(t2)

            # ---- C: back to real domain, SiLU * up ----
            h = []
            for q in range(11):
                for mt in range(4):
                    pg = ps.tile([P, HCHUNK], f32, tag="tc")
                    for kt in range(4):
                        nc.tensor.matmul(
                            pg, ti_sb[:, kt, mt, :], gH[q * 4 + kt],
                            start=(kt == 0), stop=(kt == 3),
                        )
                    sg = sp.tile([P, HCHUNK], f32, tag="sg")
                    nc.scalar.activation(sg, pg, mybir.ActivationFunctionType.Silu)
                    pu = ps.tile([P, HCHUNK], f32, tag="tc")
                    for kt in range(4):
                        nc.tensor.matmul(
                            pu, ti_sb[:, kt, mt, :], uH[q * 4 + kt],
                            start=(kt == 0), stop=(kt == 3),
                        )
                    t_ = ap_.tile([P, HCHUNK], cdt, tag="act")
                    nc.vector.tensor_mul(t_, sg, pu)
                    h.append(t_)

            # ---- D: Hh = H @ h per q-block ----
            hH = []
            for q in range(11):
                for kt in range(4):
                    pa = ps.tile([P, HCHUNK], f32, tag="ta")
                    for kin in range(4):
                        nc.tensor.matmul(
                            pa, tf_sb[:, kin, kt, :], h[q * 4 + kin],
                            start=(kin == 0), stop=(kin == 3),
                        )
                    t_ = ap_.tile([P, HCHUNK], cdt, tag="act")
                    if q % 2 == 0:
                        nc.scalar.copy(t_, pa)
                    else:
                        nc.vector.tensor_copy(t_, pa)
                    hH.append(t_)

            # ---- E: mix down ----
            dH = []
            for p2 in range(4):
                for kt in range(4):
                    o = p2 * 4 + kt
                    wd_sb = mwp.tile([P, 11, P], cdt, tag="md")
                    nc.sync.dma_start(out=wd_sb, in_=md[o].rearrange("p (a m) -> p a m", a=11))
                    pb = ps.tile([P, HCHUNK], f32, tag="tb")
                    for q in range(11):
                        nc.tensor.matmul(
                            pb, wd_sb[:, q, :], hH[q * 4 + kt],
                            start=(q == 0), stop=(q == 10),
                        )
                    t_ = ap_.tile([P, HCHUNK], cdt, tag="act")
                    nc.vector.tensor_copy(t_, pb)
                    dH.append(t_)

            # ---- F: iH -> real output ----
            for p2 in range(4):
                for mt in range(4):
                    pf = ps.tile([P, HCHUNK], f32, tag="td")
                    for kt in range(4):
                        nc.tensor.matmul(
                            pf, ti_sb[:, kt, mt, :], dH[p2 * 4 + kt],
                            start=(kt == 0), stop=(kt == 3),
                        )
                    o_sb = op.tile([P, HCHUNK], f32, tag="o")
                    nc.vector.tensor_copy(o_sb, pf)
                    nc.sync.dma_start(
                        out=out[p2 * 4 + mt][:, ic * HCHUNK:(ic + 1) * HCHUNK], in_=o_sb
                    )

    nc.finalize()
    _built["hart"] = nc
    return nc


def _hart_in_maps(x, w_gate, w_up, w_down):
    T_f, T_i, R = _hartley_mats()
    bf = ml_dtypes.bfloat16

    tf_pack = np.ascontiguousarray(
        T_f.reshape(4, P, 4, P).transpose(1, 0, 2, 3)
    ).reshape(P, 16 * P).astype(bf)
    ti_pack = np.ascontiguousarray(
        T_i.reshape(4, P, 4, P).transpose(1, 0, 2, 3)
    ).reshape(P, 16 * P).astype(bf)

    tg = _mix_tiles(np.asarray(w_gate, np.float32), R)   # [11,4,4,128,128]
    tu = _mix_tiles(np.asarray(w_up, np.float32), R)
    td = _mix_tiles(np.asarray(w_down, np.float32), R)   # [4,11,4,128,128]
    mg_pack = np.ascontiguousarray(tg.transpose(0, 2, 3, 1, 4)).reshape(44, P, 4 * P).astype(bf)
    mu_pack = np.ascontiguousarray(tu.transpose(0, 2, 3, 1, 4)).reshape(44, P, 4 * P).astype(bf)
    md_pack = np.ascontiguousarray(td.transpose(0, 2, 3, 1, 4)).reshape(16, P, 11 * P).astype(bf)

    xf = np.asarray(x, np.float32).reshape(TOK_TOTAL, D_MODEL)
    in_maps = []
    for c in range(N_CORES):
        xc = xf[c * TOK_CORE:(c + 1) * TOK_CORE]
        xt = np.ascontiguousarray(
            xc.reshape(N_HCH, HCHUNK, KT, P).transpose(0, 3, 2, 1)
        ).reshape(N_HCH, P, KT * HCHUNK).astype(bf)
        in_maps.append({
            "xT": xt, "tf": tf_pack, "ti": ti_pack,
            "mg": mg_pack, "mu": mu_pack, "md": md_pack,
        })
    return in_maps


def _materialize(w):
    """(q, p, b) circulant generators -> dense [p*b, q*b] (in-dim, out-dim)."""
    q, p, b = w.shape
    i = np.arange(b)
    idx = (i[None, :] - i[:, None]) % b          # [j, i]
    return w[:, :, idx].transpose(1, 2, 0, 3).reshape(p * b, q * b)


def kernel(x, w_gate, w_up, w_down):
    mode = MODE
    if mode == "hart":
        nc = _build_hart()
        in_maps = _hart_in_maps(x, w_gate, w_up, w_down)
        return _run(nc, in_maps)
    cdt, npdt, pass_t, mm_n, _ = _MODE_CFG[mode]
    n_pass = TOK_CORE // pass_t

    nc = _build(mode)

    Wg = _materialize(np.asarray(w_gate, np.float32))   # [2048, 5632]
    Wu = _materialize(np.asarray(w_up, np.float32))     # [2048, 5632]
    Wd = _materialize(np.asarray(w_down, np.float32))   # [5632, 2048]

    # wgu packed: [MT, P, 2, KT, P]; per-partition rows contiguous
    wgu = np.empty((MT, P, 2, KT, P), np.float32)
    wg4 = Wg.reshape(KT, P, MT, P)   # [k, kp, m, mp]
    wu4 = Wu.reshape(KT, P, MT, P)
    wgu[:, :, 0] = wg4.transpose(2, 1, 0, 3)  # [m, kp, k, mp]
    wgu[:, :, 1] = wu4.transpose(2, 1, 0, 3)
    wgu = wgu.reshape(MT, P, 2 * KT * P).astype(npdt)

    wd4 = Wd.reshape(MT, P, KT, P)   # [k2, kp, m2, mp]
    wdp = np.ascontiguousarray(wd4.transpose(2, 1, 0, 3)).reshape(KT, P, MT * P).astype(npdt)

    xf = np.asarray(x, np.float32).reshape(TOK_TOTAL, D_MODEL)
    in_maps = []
    for c in range(N_CORES):
        xc = xf[c * TOK_CORE:(c + 1) * TOK_CORE]          # [2048 tok, 2048 d]
        # -> [n_pass, P, KT, pass_t]: xT[pass, kp, k, t] = xc[pass*pt+t, k*P+kp]
        xt = np.ascontiguousarray(
            xc.reshape(n_pass, pass_t, KT, P).transpose(0, 3, 2, 1)
        ).reshape(n_pass, P, KT * pass_t).astype(npdt)
        in_maps.append({"xT": xt, "wgu": wgu, "wd": wdp})

    return _run(nc, in_maps)


def _run(nc, in_maps):
    trace = bool(os.environ.get("BASS_PROFILE"))
    res = run_bass_kernel_spmd(nc, in_maps, core_ids=list(range(N_CORES)), trace=trace)
    global last_results
    last_results = res

    out = np.empty((TOK_TOTAL, D_MODEL), np.float32)
    for c in range(N_CORES):
        o = res.results[c]["outT"]                         # [KT, P, TOK_CORE]
        out[c * TOK_CORE:(c + 1) * TOK_CORE] = o.reshape(D_MODEL, TOK_CORE).T
    return out.reshape(4, 4096, D_MODEL)
